# revision 4
# baseline (speedup 1.0000x reference)
"""Self-contained Bass/Trainium2 kernel for the 2-layer LSTM + linear head.

Problem: x [2048, 512, 8] -> 2-layer LSTM (H=50, PyTorch gate order i,f,g,o)
-> last hidden state of layer 2 -> linear [1, 50] -> y [2048, 1].

Strategy: pure data parallel over 8 NeuronCores (256 batch rows each). On
each core the batch is further split into two independent 128-wide
sub-batch pipelines so the serial T=512 recurrence latency is overlapped.

Per-core layout (per sub-batch sb, 128 batch columns on the free dim):
  - Gate rows on partitions, padded to 64-row blocks so every engine access
    starts at a 32-aligned partition base: chunk A = [i rows 0:50 | f rows
    64:114], chunk B = [g rows 0:50 | o rows 64:114]. The g block's weights
    are pre-scaled by 2 so tanh(z) = 2*sigmoid(2z) - 1 needs only sigmoid.
  - One rhs tile R [128, 128] per step: rows 0:50 h0, 50:58 x_t, 58 ones
    (bias row), 59:64 zeros, 64:114 h1 (layer 2 runs one step behind layer
    1 - the skew makes both layers' gate matmuls computable in the same
    iteration). Biases are folded into the matmul via the ones row.
  - 4 matmuls per sb per step into one PSUM tile g [128, 512] (cols
    A-L0 | A-L1 | B-L0 | B-L1), one sigmoid over all gates, then the cell
    update on VectorE: u' = 2*i*sig2g (fused scalar_tensor_tensor),
    t1 = u' - i  (so t1 = i*tanh(zg)), v = f*c, c' = t1 + v, tanh(c') on
    ScalarE, h0/h1 products written straight into the next step's rhs tile.
"""
import numpy as np
import concourse.bacc as bacc
import concourse.mybir as mybir
from concourse.tile import TileContext
from concourse.bass_utils import run_bass_kernel_spmd

f32 = mybir.dt.float32
AF = mybir.ActivationFunctionType
ALU = mybir.AluOpType

H = 50
D = 8
B = 2048
T = 512
NCORES = 8
BC = B // NCORES   # 256 batch rows per core
NSB = 2
SB = BC // NSB     # 128 batch cols per sub-batch

_NC_CACHE = {}


def _build_nc(repeat=1):
    nc = bacc.Bacc(None, target_bir_lowering=False)

    xT = nc.dram_tensor("xT", [T, 14, BC], f32, kind="ExternalInput")
    w0a = nc.dram_tensor("w0a", [59, 128], f32, kind="ExternalInput")
    w0b = nc.dram_tensor("w0b", [59, 128], f32, kind="ExternalInput")
    w1a = nc.dram_tensor("w1a", [115, 128], f32, kind="ExternalInput")
    w1b = nc.dram_tensor("w1b", [115, 128], f32, kind="ExternalInput")
    wfin = nc.dram_tensor("wfin", [128, 1], f32, kind="ExternalInput")
    y = nc.dram_tensor("y", [1, BC], f32, kind="ExternalOutput")

    with TileContext(nc) as tc:
        with (
            tc.tile_pool(name="wp", bufs=1) as wp,
            tc.tile_pool(name="st", bufs=1) as st,
            tc.tile_pool(name="rp", bufs=3) as rp,
            tc.tile_pool(name="sp", bufs=2) as sp,
            tc.tile_pool(name="tp", bufs=2) as tp,
            tc.tile_pool(name="gp", bufs=2, space="PSUM") as gp,
        ):
            W0A = wp.tile([59, 128], f32, name="W0A")
            W0B = wp.tile([59, 128], f32, name="W0B")
            W1A = wp.tile([115, 128], f32, name="W1A")
            W1B = wp.tile([115, 128], f32, name="W1B")
            WF = wp.tile([128, 1], f32, name="WF")
            nc.sync.dma_start(out=W0A, in_=w0a[:, :])
            nc.sync.dma_start(out=W0B, in_=w0b[:, :])
            nc.sync.dma_start(out=W1A, in_=w1a[:, :])
            nc.sync.dma_start(out=W1B, in_=w1b[:, :])
            nc.sync.dma_start(out=WF, in_=wfin[:, :])

            for _rep in range(repeat):
                _body(nc, tc, st, rp, sp, tp, gp, xT, W0A, W0B, W1A, W1B, WF, y)

    nc.compile()
    return nc


def _body(nc, tc, st, rp, sp, tp, gp, xT, W0A, W0B, W1A, W1B, WF, y):
    if True:
        if True:
            C = [st.tile([128, 256], f32, name=f"C{sb}") for sb in range(NSB)]
            TH = [st.tile([128, 256], f32, name=f"TH{sb}") for sb in range(NSB)]
            for sb in range(NSB):
                nc.vector.memset(C[sb], 0.0)

            def new_r(sb, t, memset):
                r = rp.tile([128, SB], f32, name=f"rt{sb}", tag=f"r_{sb}")
                if memset:
                    nc.vector.memset(r, 0.0)
                nc.sync.dma_start(out=r[50:64, :],
                                  in_=xT[min(t, T - 1)][:, sb * SB:(sb + 1) * SB])
                return r

            rcur = [new_r(sb, 0, True) for sb in range(NSB)]

            for t in range(T + 1):
                rnext = [new_r(sb, t + 1, t + 1 <= 2) for sb in range(NSB)]
                g = [gp.tile([128, 512], f32, name=f"g{sb}", tag=f"g{sb}")
                     for sb in range(NSB)]
                for sb in range(NSB):
                    nc.tensor.matmul(g[sb][:, 0:128], W0A[0:59, :],
                                     rcur[sb][0:59, :], start=True, stop=True)
                for sb in range(NSB):
                    nc.tensor.matmul(g[sb][:, 256:384], W0B[0:59, :],
                                     rcur[sb][0:59, :], start=True, stop=True)
                for sb in range(NSB):
                    nc.tensor.matmul(g[sb][:, 128:256], W1A[0:114, :],
                                     rcur[sb][0:114, :], start=True, stop=True)
                for sb in range(NSB):
                    nc.tensor.matmul(g[sb][:, 384:512], W1B[0:114, :],
                                     rcur[sb][0:114, :], start=True, stop=True)

                for sb in range(NSB):
                    s = sp.tile([128, 512], f32, name=f"s{sb}", tag=f"s{sb}")
                    nc.scalar.activation(out=s, in_=g[sb][:, :], func=AF.Sigmoid)

                    up = tp.tile([128, 256], f32, name=f"up{sb}", tag=f"up{sb}")
                    t1 = tp.tile([128, 256], f32, name=f"t1{sb}", tag=f"t1{sb}")
                    v = tp.tile([128, 256], f32, name=f"v{sb}", tag=f"v{sb}")
                    # u' = (sig_2g * 2) * i
                    nc.vector.scalar_tensor_tensor(out=up[0:64, :],
                                                   in0=s[0:64, 256:512],
                                                   scalar=2.0, in1=s[0:64, 0:256],
                                                   op0=ALU.mult, op1=ALU.mult)
                    # t1 = u' - i = i * tanh(zg)
                    nc.vector.tensor_tensor(out=t1[64:128, :], in0=up[0:64, :],
                                            in1=s[0:64, 0:256], op=ALU.subtract)
                    # v = f * c
                    nc.vector.tensor_tensor(out=v[64:128, :], in0=s[64:128, 0:256],
                                            in1=C[sb][64:128, :], op=ALU.mult)
                    # c' = t1 + v
                    nc.vector.tensor_tensor(out=C[sb][64:128, :], in0=t1[64:128, :],
                                            in1=v[64:128, :], op=ALU.add)
                    # th = tanh(c')
                    nc.scalar.activation(out=TH[sb][64:128, :], in_=C[sb][64:128, :],
                                         func=AF.Tanh)
                    # h = o * th; layer-1 half feeds rows 0:50, layer-2 rows 64:114
                    nc.vector.tensor_tensor(out=rnext[sb][0:50, :],
                                            in0=s[64:114, 256:384],
                                            in1=TH[sb][64:114, 0:128], op=ALU.mult)
                    nc.vector.tensor_tensor(out=rnext[sb][64:114, :],
                                            in0=s[64:114, 384:512],
                                            in1=TH[sb][64:114, 128:256], op=ALU.mult)

                if t == 0:
                    # layer 2 ran on junk at t=0 (its real step 0 happens at t=1)
                    for sb in range(NSB):
                        nc.vector.memset(C[sb][64:128, 128:256], 0.0)
                        nc.vector.memset(rnext[sb][64:114, :], 0.0)
                rcur = rnext

            ysb = st.tile([1, BC], f32, name="ysb")
            for sb in range(NSB):
                fin = gp.tile([1, SB], f32, name=f"fin{sb}", tag=f"g{sb}")
                nc.tensor.matmul(fin[:, :], WF[64:114, :], rcur[sb][64:114, :],
                                 start=True, stop=True)
                nc.scalar.copy(out=ysb[:, sb * SB:(sb + 1) * SB], in_=fin[:, :])
            nc.sync.dma_start(out=y[:, :], in_=ysb)


def _prep_weights(Wih0, Whh0, bih0, bhh0, Wih1, Whh1, bih1, bhh1):
    """Stacked/padded lhsT blobs; biases in K-row 58 (the rhs ones row)."""
    b0 = (np.asarray(bih0) + np.asarray(bhh0)).astype(np.float32)
    b1 = (np.asarray(bih1) + np.asarray(bhh1)).astype(np.float32)

    def chunk(hrows, xrows, onerow, Wh, Wx, b, g0, g1, krows, sc0=1.0, sc1=1.0):
        out = np.zeros((krows, 128), dtype=np.float32)
        for col0, gi, sc in ((0, g0, sc0), (64, g1, sc1)):
            rows = slice(gi * H, (gi + 1) * H)
            out[hrows, col0:col0 + H] = np.asarray(Wh)[rows, :].T * sc
            out[xrows, col0:col0 + H] = np.asarray(Wx)[rows, :].T * sc
            out[onerow, col0:col0 + H] = b[rows] * sc
        return out

    w0a = chunk(slice(0, 50), slice(50, 58), 58, Whh0, Wih0, b0, 0, 1, 59)
    w0b = chunk(slice(0, 50), slice(50, 58), 58, Whh0, Wih0, b0, 2, 3, 59, 2.0, 1.0)
    w1a = chunk(slice(64, 114), slice(0, 50), 58, Whh1, Wih1, b1, 0, 1, 115)
    w1b = chunk(slice(64, 114), slice(0, 50), 58, Whh1, Wih1, b1, 2, 3, 115, 2.0, 1.0)
    return w0a, w0b, w1a, w1b


def _make_in_maps(x, Wih0, Whh0, bih0, bhh0, Wih1, Whh1, bih1, bhh1,
                  Wlin, blin):
    x = np.asarray(x, dtype=np.float32)
    w0a, w0b, w1a, w1b = _prep_weights(Wih0, Whh0, bih0, bhh0,
                                       Wih1, Whh1, bih1, bhh1)
    wfin = np.zeros((128, 1), np.float32)
    wfin[64:114, 0] = np.asarray(Wlin, dtype=np.float32)[0, :]

    in_maps = []
    for c in range(NCORES):
        xc = x[c * BC:(c + 1) * BC]              # [BC, T, D]
        xt = np.zeros((T, 14, BC), dtype=np.float32)
        xt[:, 0:D, :] = xc.transpose(1, 2, 0)
        xt[:, D, :] = 1.0                        # ones row (bias)
        in_maps.append({"xT": xt, "w0a": w0a, "w0b": w0b, "w1a": w1a,
                        "w1b": w1b, "wfin": wfin})
    return in_maps


def kernel(x, Wih0, Whh0, bih0, bhh0, Wih1, Whh1, bih1, bhh1, Wlin, blin):
    in_maps = _make_in_maps(x, Wih0, Whh0, bih0, bhh0, Wih1, Whh1,
                            bih1, bhh1, Wlin, blin)
    if "nc" not in _NC_CACHE:
        _NC_CACHE["nc"] = _build_nc()
    nc = _NC_CACHE["nc"]

    res = run_bass_kernel_spmd(nc, in_maps, core_ids=list(range(NCORES)))
    out = np.empty((B, 1), dtype=np.float32)
    blin_v = np.float32(np.asarray(blin).reshape(-1)[0])
    for c in range(NCORES):
        out[c * BC:(c + 1) * BC, 0] = res.results[c]["y"][0] + blin_v
    return out



# revision 13
# speedup vs baseline: 3.0821x; 3.0821x over previous
"""Self-contained Bass/Trainium2 kernel for the 2-layer LSTM + linear head.

Problem: x [2048, 512, 8] -> 2-layer LSTM (H=50, PyTorch gate order i,f,g,o)
-> last hidden state of layer 2 -> linear [1, 50] -> y [2048, 1].

Strategy: pure data parallel over 8 NeuronCores (256 batch rows each). On
each core the batch is further split into NSB independent sub-batch
pipelines (chains) so the serial T=512 recurrence latency is overlapped.

v5: full bf16 datapath (CPU emulation: rel err ~5e-3 vs the 2e-2 gate).

Gate-per-chunk layout: each of the four matmul chunks holds ONE gate for
BOTH layers - layer 0 on partitions 0:64 (50 used), layer 1 on partitions
64:128.  Both layers share one rhs (lhsT columns pick the rows each layer
reads: L0 cols read h0/x rows with Whh0/Wih0, L1 cols read h0 rows with
Wih1 and h1 rows with Whh1), so the per-step elementwise phase operates on
fully-dense [128, w] tiles: one stt for m = (sig(2zg)-0.5)*i, one mult for
v = f*ct, one add for ct', one tanh (scale=2; cell state is stored halved)
and ONE h = o*th product covering both layers (junk pad rows 50:64 are
multiplied by zero weight columns in the next matmul).

x / bias contributions enter the gate PSUM via separate K=9 matmuls from a
staging tile DMA-loaded 8 timesteps at a time (ones row 8 carries both
layers' biases), so no DMA sits near the serial path.  Layer 2 runs one
step behind layer 1 (skew) which makes both layers' matmuls computable in
the same iteration.
"""
import numpy as np
import ml_dtypes
import concourse.bacc as bacc
import concourse.mybir as mybir
from concourse.tile import TileContext
from concourse.bass_utils import run_bass_kernel_spmd

f32 = mybir.dt.float32
bf16 = mybir.dt.bfloat16
AF = mybir.ActivationFunctionType
ALU = mybir.AluOpType

H = 50
D = 8
B = 2048
T = 512
NCORES = 8
BC = B // NCORES   # 256 batch rows per core
NSB = 3
_w = BC // NSB
SBS = [_w + (1 if i < BC - _w * NSB else 0) for i in range(NSB)]
OFFS = [sum(SBS[:i]) for i in range(NSB)]
XG = 8             # timesteps per x-staging DMA
NG = (T + 1 + XG - 1) // XG

GATES = ("i", "f", "g", "o")

_NC_CACHE = {}


def _build_nc(repeat=1):
    nc = bacc.Bacc(None, target_bir_lowering=False)

    xT = nc.dram_tensor("xT", [9, T, BC], bf16, kind="ExternalInput")
    wh = {}
    for q in GATES:
        wh[f"h{q}"] = nc.dram_tensor(f"wh{q}", [114, 128], bf16,
                                     kind="ExternalInput")
        wh[f"x{q}"] = nc.dram_tensor(f"wx{q}", [9, 128], bf16,
                                     kind="ExternalInput")
    wfin = nc.dram_tensor("wfin", [128, 1], bf16, kind="ExternalInput")
    y = nc.dram_tensor("y", [1, BC], f32, kind="ExternalOutput")

    with TileContext(nc) as tc:
        with (
            tc.tile_pool(name="wp", bufs=1) as wp,
            tc.tile_pool(name="st", bufs=1) as st,
            tc.tile_pool(name="rp", bufs=3) as rp,
            tc.tile_pool(name="xp", bufs=2) as xp,
            tc.tile_pool(name="sp", bufs=2) as sp,
            tc.tile_pool(name="tp", bufs=2) as tp,
            tc.tile_pool(name="gp", bufs=2, space="PSUM") as gp,
        ):
            W = {}
            for k, dt in wh.items():
                W[k] = wp.tile(list(dt.shape), bf16, name=f"W{k}")
                nc.sync.dma_start(out=W[k], in_=dt[:, :])
            WF = wp.tile([128, 1], bf16, name="WF")
            nc.sync.dma_start(out=WF, in_=wfin[:, :])

            for _rep in range(repeat):
                _lstm_body(nc, st, rp, xp, sp, tp, gp, xT, W, WF, y)

    nc.compile()
    return nc


def _lstm_body(nc, st, rp, xp, sp, tp, gp, xT, W, WF, y):
    C = [st.tile([128, SBS[sb]], bf16, name=f"C{sb}") for sb in range(NSB)]
    TH = [st.tile([128, SBS[sb]], bf16, name=f"TH{sb}") for sb in range(NSB)]
    for sb in range(NSB):
        nc.vector.memset(C[sb], 0.0)

    def new_r(sb, memset):
        r = rp.tile([128, SBS[sb]], bf16, name=f"rt{sb}", tag=f"r_{sb}")
        if memset:
            nc.vector.memset(r, 0.0)
        return r

    def load_group(sb, gidx):
        t0 = min(gidx * XG, T - XG)
        xs = xp.tile([16, XG, SBS[sb]], bf16, name=f"xs{sb}", tag=f"xs{sb}")
        nc.sync.dma_start(
            out=xs[0:9, :, :],
            in_=xT[0:9, t0:t0 + XG, OFFS[sb]:OFFS[sb] + SBS[sb]])
        return xs

    rcur = [new_r(sb, True) for sb in range(NSB)]
    xs_tiles = {0: [load_group(sb, 0) for sb in range(NSB)],
                1: [load_group(sb, 1) for sb in range(NSB)]}

    def xside_mms(g, t):
        # x/bias contribution for step t (start=True clears PSUM); these
        # only depend on the staging tile, so they run ahead of the h-side.
        gidx = min(t // XG, NG - 1)
        slot = min(t - min(gidx * XG, T - XG), XG - 1)
        xs = xs_tiles[gidx]
        for qi in range(4):
            q = GATES[qi]
            for sb in range(NSB):
                w = SBS[sb]
                nc.tensor.matmul(g[sb][:, qi * w:(qi + 1) * w],
                                 W[f"x{q}"][:, :], xs[sb][0:9, slot, :],
                                 start=True, stop=False)

    def new_g():
        return [gp.tile([128, 4 * SBS[sb]], f32, name=f"g{sb}", tag=f"g{sb}")
                for sb in range(NSB)]

    g = new_g()
    xside_mms(g, 0)

    for t in range(T + 1):
        if t > 0 and t % XG == 0:
            gi = t // XG + 1
            if gi < NG:
                xs_tiles[gi] = [load_group(sb, gi) for sb in range(NSB)]
                xs_tiles.pop(gi - 2, None)

        rnext = [new_r(sb, t + 1 <= 2) for sb in range(NSB)]
        for qi in range(4):
            q = GATES[qi]
            for sb in range(NSB):
                w = SBS[sb]
                nc.tensor.matmul(g[sb][:, qi * w:(qi + 1) * w],
                                 W[f"h{q}"][:, :], rcur[sb][0:114, :],
                                 start=False, stop=True)

        gprev, g = g, (new_g() if t < T else None)
        if g is not None:
            xside_mms(g, t + 1)

        for sb in range(NSB):
            w = SBS[sb]
            s = sp.tile([128, 4 * w], bf16, name=f"s{sb}", tag=f"s{sb}")
            nc.scalar.activation(out=s, in_=gprev[sb][:, :], func=AF.Sigmoid)

            m = tp.tile([128, w], bf16, name=f"m{sb}", tag=f"m{sb}")
            v = tp.tile([128, w], bf16, name=f"v{sb}", tag=f"v{sb}")
            # m = (sig2g - 0.5) * i = i*tanh(zg)/2   (DVE stt)
            nc.vector.scalar_tensor_tensor(out=m, in0=s[:, 2 * w:3 * w],
                                           scalar=0.5, in1=s[:, 0:w],
                                           op0=ALU.subtract, op1=ALU.mult)
            # v = f * ct    (ct = c/2 cell state)
            nc.vector.tensor_tensor(out=v, in0=s[:, w:2 * w],
                                    in1=C[sb], op=ALU.mult)
            # ct' = m + v
            nc.vector.tensor_tensor(out=C[sb], in0=m, in1=v, op=ALU.add)
            # th = tanh(2*ct')
            nc.scalar.activation(out=TH[sb], in_=C[sb], func=AF.Tanh,
                                 scale=2.0)
            # h = o * th, both layers at once (junk pad rows are harmless:
            # they hit zero-weight lhsT columns in the next matmul)
            nc.vector.tensor_tensor(out=rnext[sb][0:114, :],
                                    in0=s[0:114, 3 * w:4 * w],
                                    in1=TH[sb][0:114, :], op=ALU.mult)

        if t == 0:
            # layer 2 ran on junk at t=0 (its real step 0 happens at t=1)
            for sb in range(NSB):
                nc.vector.memset(C[sb][64:128, :], 0.0)
                nc.vector.memset(rnext[sb][64:114, :], 0.0)
        rcur = rnext

    ysb = st.tile([1, BC], f32, name="ysb")
    for sb in range(NSB):
        w = SBS[sb]
        fin = gp.tile([1, w], f32, name=f"fin{sb}", tag=f"g{sb}")
        nc.tensor.matmul(fin[:, :], WF[64:114, :], rcur[sb][64:114, :],
                         start=True, stop=True)
        nc.scalar.copy(out=ysb[:, OFFS[sb]:OFFS[sb] + w], in_=fin[:, :])
    nc.sync.dma_start(out=y[:, :], in_=ysb)


def _prep_weights(Wih0, Whh0, bih0, bhh0, Wih1, Whh1, bih1, bhh1):
    """Per-gate lhsT blobs (bf16), both layers in one 128-col tile.

    h-side wh{q} [114,128]: L0 cols 0:50 read h0 rows (Whh0^T); L1 cols
    64:114 read h0 rows 0:50 (Wih1^T) and h1 rows 64:114 (Whh1^T).
    x-side wx{q} [9,128]: rows 0:8 = Wih0^T (L0 cols only), row 8 = bias
    for both layers.  The g gate is pre-scaled by 2 (tanh-via-sigmoid).
    """
    b0 = (np.asarray(bih0) + np.asarray(bhh0)).astype(np.float32)
    b1 = (np.asarray(bih1) + np.asarray(bhh1)).astype(np.float32)
    Wih0 = np.asarray(Wih0); Whh0 = np.asarray(Whh0)
    Wih1 = np.asarray(Wih1); Whh1 = np.asarray(Whh1)

    out = {}
    for qi, q in enumerate(GATES):
        sc = 2.0 if q == "g" else 1.0
        rows = slice(qi * H, (qi + 1) * H)
        whq = np.zeros((114, 128), np.float32)
        whq[0:50, 0:50] = Whh0[rows, :].T * sc
        whq[0:50, 64:114] = Wih1[rows, :].T * sc
        whq[64:114, 64:114] = Whh1[rows, :].T * sc
        wxq = np.zeros((9, 128), np.float32)
        wxq[0:8, 0:50] = Wih0[rows, :].T * sc
        wxq[8, 0:50] = b0[rows] * sc
        wxq[8, 64:114] = b1[rows] * sc
        out[f"wh{q}"] = whq.astype(ml_dtypes.bfloat16)
        out[f"wx{q}"] = wxq.astype(ml_dtypes.bfloat16)
    return out


def _make_in_maps(x, Wih0, Whh0, bih0, bhh0, Wih1, Whh1, bih1, bhh1,
                  Wlin, blin):
    x = np.asarray(x, dtype=np.float32)
    wd = _prep_weights(Wih0, Whh0, bih0, bhh0, Wih1, Whh1, bih1, bhh1)
    wfin = np.zeros((128, 1), np.float32)
    wfin[64:114, 0] = np.asarray(Wlin, dtype=np.float32)[0, :]
    wfin = wfin.astype(ml_dtypes.bfloat16)

    in_maps = []
    for c in range(NCORES):
        xc = x[c * BC:(c + 1) * BC]              # [BC, T, D]
        xt = np.zeros((9, T, BC), dtype=np.float32)
        xt[0:D] = xc.transpose(2, 1, 0)
        xt[D] = 1.0                              # ones row (bias)
        im = {"xT": xt.astype(ml_dtypes.bfloat16), "wfin": wfin}
        im.update(wd)
        in_maps.append(im)
    return in_maps


def kernel(x, Wih0, Whh0, bih0, bhh0, Wih1, Whh1, bih1, bhh1, Wlin, blin):
    in_maps = _make_in_maps(x, Wih0, Whh0, bih0, bhh0, Wih1, Whh1,
                            bih1, bhh1, Wlin, blin)
    if "nc" not in _NC_CACHE:
        _NC_CACHE["nc"] = _build_nc()
    nc = _NC_CACHE["nc"]

    res = run_bass_kernel_spmd(nc, in_maps, core_ids=list(range(NCORES)))
    out = np.empty((B, 1), dtype=np.float32)
    blin_v = np.float32(np.asarray(blin).reshape(-1)[0])
    for c in range(NCORES):
        out[c * BC:(c + 1) * BC, 0] = res.results[c]["y"][0] + blin_v
    return out
